# revision 13
# baseline (speedup 1.0000x reference)
"""Trainium2 Bass kernel for the Clusteror GNN message-passing block.

Full (unsharded) inputs in, full output out. Internally: data-parallel over
node rows across 8 NeuronCores; the P=10 virtual-node path is precomputed on
host (it is tiny and feeds every row via a gather, which we lower to a
one-hot matmul).

Math notes (host-side rewrites):
 - LayerNorm mean removal is folded into the weights: column-centered W makes
   every row of z = x@Wc have exactly zero channel-mean, so the device only
   needs the second moment. rstd = exp(-0.5*ln(ssq/C + eps)) keeps all ACT
   work inside one activation-table set (ln+exp+square+identity).
 - The per-row sum of squares is computed with a ones-MATRIX matmul, which
   lands in PSUM already broadcast across all 128 partitions.
 - ELU is computed in a shifted basis: elu(x) + 1 = max(x,0) + min(exp(x),1).
   The -1 shift is folded into the next layer's bias via column sums of its
   weights.
 - The irregular gather v_x[mapping] becomes onehot(mapping) @ (v_x @ Wa2c),
   a K=10 matmul against a host-built one-hot operand.
"""

import json
import os
import tempfile
from pathlib import Path

import numpy as np

N = 200000
P = 10
C = 256
OUT = 64
NCORES = 8
RPC = N // NCORES          # 25000 real rows per core
TILE = int(os.environ.get("KERNEL_TILE", "512"))
NT = (RPC + TILE - 1) // TILE
RPAD = NT * TILE           # 25088 padded rows per core (for TILE in {256,512})
KC = 2                     # 256 channels = 2 partition chunks of 128
EPS = 1e-5

# dtype config: "bf16" (fast) or "f32r" (precise).
MM_MODE = os.environ.get("KERNEL_MM_MODE", "bf16")
PSZ = int(os.environ.get("KERNEL_PSZ", "2"))
PSS = int(os.environ.get("KERNEL_PSS", "2"))
PSY = int(os.environ.get("KERNEL_PSY", "2"))
ABUFS = int(os.environ.get("KERNEL_ABUFS", "3"))

_CACHE = {}
_LAST_IN_MAPS = None


def _np_layer_norm(x, g, b, eps=EPS):
    mu = x.mean(-1, keepdims=True)
    var = x.var(-1, keepdims=True)
    return (x - mu) / np.sqrt(var + eps) * g + b


def _np_elu(x):
    return np.where(x > 0, x, np.expm1(np.minimum(x, 0.0)))


def _ensure_act_root():
    """Reorder act_info.json so natural_log_exp_and_others (which covers
    square+ln+exp+identity+copy) is set 0, and make BOTH bass (which
    pre-places InstLoadActFuncSet with a set id) and walrus (which adopts
    those ids against --act-root-json) see the same reordered file. With a
    covering set first, every ACTIVATE resolves to one table."""
    cur = os.environ.get("BASS_ACT_ROOT_JSON_PATH", "")
    if cur and "_ntbl" in cur:
        return
    import neuronxcc

    src = Path(neuronxcc.__path__[0]) / "pwp" / "pwp_bin_trainium"
    info = json.load(open(src / "act_info.json"))
    sets = info["act_func_sets"]
    tgt = [s for s in sets if s["name"] == "natural_log_exp_and_others"]
    if not tgt:
        return
    info["act_func_sets"] = tgt + [s for s in sets if s is not tgt[0]]
    d = Path(tempfile.mkdtemp(prefix="actroot_ntbl"))
    for f in src.iterdir():
        if f.name != "act_info.json":
            (d / f.name).symlink_to(f)
    act_path = d / "act_info.json"
    with open(act_path, "w") as fh:
        json.dump(info, fh)
    os.environ["BASS_ACT_ROOT_JSON_PATH"] = str(act_path)

    # bass side: get_activation_tables reads the stock act_info via
    # findActInfoFile (cached); repoint it at the reordered file so the
    # pre-placed set ids match what walrus will read.
    import functools

    import concourse.bacc as bacc_mod
    import concourse.hw_specs as hw_specs
    from concourse import mybir

    @functools.cache
    def _tables(module_arch):
        with open(act_path) as af:
            ai = json.load(af)
        return {
            ent["name"]: {
                mybir.ActivationFunctionType.from_pwp(v) for v in ent["act"].keys()
            }
            for ent in ai["act_func_sets"]
        }

    hw_specs.get_activation_tables = _tables
    bacc_mod.get_activation_tables = _tables


def _build(affine1, affine3, b1_zero, mode):
    import concourse.bacc as bacc
    import concourse.tile as tile
    from concourse import mybir

    f32 = mybir.dt.float32
    mdt = mybir.dt.bfloat16 if mode == "bf16" else mybir.dt.float32r
    adt = f32  # bf16 elementwise falls off the DVE fast paths in this stack
    Alu = mybir.AluOpType
    Act = mybir.ActivationFunctionType

    nc = bacc.Bacc("TRN2", target_bir_lowering=False, debug=False)

    # ---- DRAM I/O ----
    xT = nc.dram_tensor("xT", [C, RPAD], mdt, kind="ExternalInput").ap()
    ohT = nc.dram_tensor("ohT", [P, RPAD], mdt, kind="ExternalInput").ap()
    w1 = nc.dram_tensor("w1", [C, C], mdt, kind="ExternalInput").ap()
    w2 = nc.dram_tensor("w2", [C, C], mdt, kind="ExternalInput").ap()
    w3 = nc.dram_tensor("w3", [C, C], mdt, kind="ExternalInput").ap()
    w4 = nc.dram_tensor("w4", [C, OUT], mdt, kind="ExternalInput").ap()
    vaT = nc.dram_tensor("vaT", [P, C], mdt, kind="ExternalInput").ap()
    b1 = nc.dram_tensor("b1", [C, 1], f32, kind="ExternalInput").ap()
    b2 = nc.dram_tensor("b2", [C, 1], f32, kind="ExternalInput").ap()
    b3 = nc.dram_tensor("b3", [C, 1], f32, kind="ExternalInput").ap()
    b4 = nc.dram_tensor("b4", [OUT, 1], f32, kind="ExternalInput").ap()
    gb1 = nc.dram_tensor("gb1", [C, 2], f32, kind="ExternalInput").ap()
    gb3 = nc.dram_tensor("gb3", [C, 2], f32, kind="ExternalInput").ap()
    onesd = nc.dram_tensor("onesd", [128, 128], mdt, kind="ExternalInput").ap()
    yT = nc.dram_tensor("yT", [OUT, RPAD], f32, kind="ExternalOutput").ap()

    with tile.TileContext(nc) as tc:
        with (
            tc.tile_pool(name="wpool", bufs=1) as wp,
            tc.tile_pool(name="xin", bufs=4) as xin,
            tc.tile_pool(name="act", bufs=ABUFS) as act,
            tc.tile_pool(name="small", bufs=4) as sm,
            tc.tile_pool(name="psz", bufs=PSZ, space="PSUM") as psz,
            tc.tile_pool(name="pss", bufs=PSS, space="PSUM") as pss,
            tc.tile_pool(name="psy", bufs=PSY, space="PSUM") as psy,
        ):
            # ---- resident weights/constants ----
            w1t = wp.tile([128, KC, C], mdt)
            nc.sync.dma_start(w1t[:], w1.rearrange("(k p) m -> p k m", p=128))
            w2t = wp.tile([128, KC, C], mdt)
            nc.sync.dma_start(w2t[:], w2.rearrange("(k p) m -> p k m", p=128))
            w3t = wp.tile([128, KC, C], mdt)
            nc.sync.dma_start(w3t[:], w3.rearrange("(k p) m -> p k m", p=128))
            w4t = wp.tile([128, KC, OUT], mdt)
            nc.sync.dma_start(w4t[:], w4.rearrange("(k p) m -> p k m", p=128))
            vat = wp.tile([P, C], mdt)
            nc.sync.dma_start(vat[:], vaT[:])
            b1t = wp.tile([128, KC], f32)
            nc.sync.dma_start(b1t[:], b1.rearrange("(k p) 1 -> p k", p=128))
            b2t = wp.tile([128, KC], f32)
            nc.sync.dma_start(b2t[:], b2.rearrange("(k p) 1 -> p k", p=128))
            b3t = wp.tile([128, KC], f32)
            nc.sync.dma_start(b3t[:], b3.rearrange("(k p) 1 -> p k", p=128))
            b4t = wp.tile([OUT, 1], f32)
            nc.sync.dma_start(b4t[:], b4[:])
            gb1t = wp.tile([128, KC, 2], f32)
            nc.sync.dma_start(gb1t[:], gb1.rearrange("(k p) a -> p k a", p=128))
            gb3t = wp.tile([128, KC, 2], f32)
            nc.sync.dma_start(gb3t[:], gb3.rearrange("(k p) a -> p k a", p=128))
            ones = wp.tile([128, 128], mdt)
            nc.sync.dma_start(ones[:], onesd[:])
            epst = wp.tile([128, 1], f32)
            nc.vector.memset(epst[:], EPS)

            def ln_elu(z, bt, b_zero, gbt, affine):
                """z: psum [128, KC, TILE] -> sbuf [128, KC, TILE] (mdt)
                holding elu(ln(z))+1."""
                zsq = act.tile([128, KC, TILE], mdt, tag="zsq")
                if b_zero:
                    nc.scalar.activation(zsq[:, :, :], z[:, :, :], Act.Square)
                else:
                    for c in range(KC):
                        nc.scalar.activation(
                            zsq[:, c, :], z[:, c, :], Act.Square,
                            bias=bt[:, c : c + 1],
                        )
                ssb = pss.tile([128, TILE], f32, tag="ssb")
                for c in range(KC):
                    nc.tensor.matmul(
                        ssb[:], ones[:], zsq[:, c, :],
                        start=(c == 0), stop=(c == KC - 1),
                    )
                # rstd = exp(-0.5 * ln(ssb/C + eps)); ln+exp share one table
                lg = sm.tile([128, TILE], f32, tag="lg")
                nc.scalar.activation(lg[:], ssb[:], Act.Ln, scale=1.0 / C, bias=epst[:])
                rstd = sm.tile([128, TILE], f32, tag="rstd")
                nc.scalar.activation(rstd[:], lg[:], Act.Exp, scale=-0.5)
                u = act.tile([128, KC, TILE], adt, tag="u")
                for c in range(KC):
                    nc.vector.scalar_tensor_tensor(
                        u[:, c, :], z[:, c, :], bt[:, c : c + 1], rstd[:],
                        op0=Alu.add, op1=Alu.mult,
                    )
                    if affine:
                        nc.vector.tensor_scalar(
                            u[:, c, :], u[:, c, :], gbt[:, c, 0:1], gbt[:, c, 1:2],
                            op0=Alu.mult, op1=Alu.add,
                        )
                e = act.tile([128, KC, TILE], adt, tag="e")
                nc.scalar.activation(e[:, :, :], u[:, :, :], Act.Exp)
                r = act.tile([128, KC, TILE], adt, tag="r")
                nc.vector.tensor_scalar(r[:, :, :], u[:, :, :], 0.0, None, op0=Alu.max)
                h = act.tile([128, KC, TILE], mdt, tag="h")
                nc.vector.scalar_tensor_tensor(
                    h[:, :, :], e[:, :, :], 1.0, r[:, :, :], op0=Alu.min, op1=Alu.add
                )
                return h

            for t in range(NT):
                cols = slice(t * TILE, (t + 1) * TILE)
                xt = xin.tile([128, KC, TILE], mdt, tag="xt")
                nc.sync.dma_start(
                    xt[:], xT[:, cols].rearrange("(k p) n -> p k n", p=128)
                )
                oht = xin.tile([P, TILE], mdt, tag="oht")
                nc.sync.dma_start(oht[:], ohT[:, cols])

                # L1
                z1 = psz.tile([128, KC, TILE], f32, tag="z")
                for m in range(KC):
                    for k in range(KC):
                        nc.tensor.matmul(
                            z1[:, m, :],
                            w1t[:, k, m * 128 : (m + 1) * 128],
                            xt[:, k, :],
                            start=(k == 0), stop=(k == KC - 1),
                        )
                h1 = ln_elu(z1, b1t, b1_zero, gb1t, affine1)

                # L2 (elu only; per-channel bias fused into ACT/DVE ops)
                z2 = psz.tile([128, KC, TILE], f32, tag="z")
                for m in range(KC):
                    for k in range(KC):
                        nc.tensor.matmul(
                            z2[:, m, :],
                            w2t[:, k, m * 128 : (m + 1) * 128],
                            h1[:, k, :],
                            start=(k == 0), stop=(k == KC - 1),
                        )
                ex = act.tile([128, KC, TILE], adt, tag="ex")
                r2 = act.tile([128, KC, TILE], adt, tag="r2")
                for c in range(KC):
                    nc.scalar.activation(
                        ex[:, c, :], z2[:, c, :], Act.Exp, bias=b2t[:, c : c + 1]
                    )
                    nc.vector.tensor_scalar(
                        r2[:, c, :], z2[:, c, :], b2t[:, c : c + 1], 0.0,
                        op0=Alu.add, op1=Alu.max,
                    )
                h2 = act.tile([128, KC, TILE], mdt, tag="h2")
                nc.vector.scalar_tensor_tensor(
                    h2[:, :, :], ex[:, :, :], 1.0, r2[:, :, :],
                    op0=Alu.min, op1=Alu.add,
                )

                # L3 (+ va gather via one-hot matmul)
                z3 = psz.tile([128, KC, TILE], f32, tag="z")
                for m in range(KC):
                    for k in range(KC):
                        nc.tensor.matmul(
                            z3[:, m, :],
                            w3t[:, k, m * 128 : (m + 1) * 128],
                            h2[:, k, :],
                            start=(k == 0), stop=False,
                        )
                    nc.tensor.matmul(
                        z3[:, m, :],
                        vat[:, m * 128 : (m + 1) * 128],
                        oht[:],
                        start=False, stop=True,
                    )
                h3 = ln_elu(z3, b3t, False, gb3t, affine3)

                # L4
                y = psy.tile([OUT, TILE], f32, tag="y")
                for k in range(KC):
                    nc.tensor.matmul(
                        y[:], w4t[:, k, :], h3[:, k, :],
                        start=(k == 0), stop=(k == KC - 1),
                    )
                yb = sm.tile([OUT, TILE], f32, tag="yb")
                nc.scalar.activation(yb[:], y[:], Act.Identity, bias=b4t[:])
                nc.sync.dma_start(yT[:, cols], yb[:])

    nc.compile()
    return nc


def _to_mm_np(a, mode):
    if mode == "bf16":
        import ml_dtypes

        return np.ascontiguousarray(a, dtype=ml_dtypes.bfloat16)
    return np.ascontiguousarray(a, np.float32)


def kernel(**inputs):
    _ensure_act_root()
    x = np.asarray(inputs["x"], np.float32)
    mapping = np.asarray(inputs["mapping"])
    vnode_embed = np.asarray(inputs["vnode_embed"], np.float32)
    vnode_bias_hid = np.asarray(inputs["vnode_bias_hid"], np.float32)
    vnode_bias_dcd = np.asarray(inputs["vnode_bias_dcd"], np.float32)
    W1 = np.asarray(inputs["W_in2hid"], np.float32)
    b1_in = np.asarray(inputs["b_in2hid"], np.float32)
    g_hid = np.asarray(inputs["g_ln_hid"], np.float32)
    b_hid = np.asarray(inputs["b_ln_hid"], np.float32)
    W2 = np.asarray(inputs["W_enc"], np.float32)
    b2_in = np.asarray(inputs["b_enc"], np.float32)
    g_enc = np.asarray(inputs["g_ln_enc"], np.float32)
    b_enc_ln = np.asarray(inputs["b_ln_enc"], np.float32)
    Wa = np.asarray(inputs["W_aggr"], np.float32)
    b_aggr = np.asarray(inputs["b_aggr"], np.float32)
    W4 = np.asarray(inputs["W_out"], np.float32)
    b_out = np.asarray(inputs["b_out"], np.float32)
    mode = MM_MODE

    # ---- host-side weight prep (float64 for the centering arithmetic) ----
    W1d = W1.astype(np.float64)
    W1c = W1d - W1d.mean(1, keepdims=True)
    b1c = b1_in.astype(np.float64) - b1_in.astype(np.float64).mean()

    W2d = W2.astype(np.float64)
    bias2 = b2_in.astype(np.float64) - W2d.sum(0)

    Wa1 = Wa[:C].astype(np.float64)
    Wa2 = Wa[C:].astype(np.float64)
    Wa1c = Wa1 - Wa1.mean(1, keepdims=True)
    Wa2c = Wa2 - Wa2.mean(1, keepdims=True)
    bad = b_aggr.astype(np.float64)
    bias3 = (bad - bad.mean()) - Wa1c.sum(0)

    W4d = W4.astype(np.float64)
    bias4 = b_out.astype(np.float64) - W4d.sum(0)

    affine1 = not (np.all(g_hid == 1.0) and np.all(b_hid == 0.0))
    affine3 = not (np.all(g_enc == 1.0) and np.all(b_enc_ln == 0.0))
    b1_zero = bool(np.all(np.asarray(b1c) == 0.0))

    # ---- virtual-node path entirely on host (10 rows) ----
    v0 = vnode_embed.astype(np.float64)
    hv = _np_elu(_np_layer_norm(v0 @ W1d + b1_in, g_hid, b_hid)) + vnode_bias_hid
    ev = _np_elu(hv @ W2d + b2_in) + vnode_bias_dcd   # true v_x [P, C]
    va_c = ev @ Wa2c                                   # centered gather payload

    # ---- shard + transpose real-node rows ----
    onehot = (mapping[None, :] == np.arange(P, dtype=mapping.dtype)[:, None])
    onehot = onehot.astype(np.float32)

    key = (affine1, affine3, b1_zero, mode, TILE, PSZ, PSS, PSY, ABUFS)
    if key not in _CACHE:
        _CACHE[key] = _build(affine1, affine3, b1_zero, mode)
    nc = _CACHE[key]

    shared = {
        "w1": _to_mm_np(W1c, mode),
        "w2": _to_mm_np(W2d, mode),
        "w3": _to_mm_np(Wa1c, mode),
        "w4": _to_mm_np(W4d, mode),
        "vaT": _to_mm_np(va_c, mode),
        "b1": np.ascontiguousarray(b1c, np.float32).reshape(C, 1),
        "b2": np.ascontiguousarray(bias2, np.float32).reshape(C, 1),
        "b3": np.ascontiguousarray(bias3, np.float32).reshape(C, 1),
        "b4": np.ascontiguousarray(bias4, np.float32).reshape(OUT, 1),
        "gb1": np.stack([g_hid, b_hid], 1).astype(np.float32),
        "gb3": np.stack([g_enc, b_enc_ln], 1).astype(np.float32),
        "onesd": _to_mm_np(np.ones((128, 128), np.float32), mode),
    }
    in_maps = []
    xTfull = x[:N].T  # [C, N]
    for c in range(NCORES):
        rows = slice(c * RPC, (c + 1) * RPC)
        xTc = np.zeros((C, RPAD), np.float32)
        xTc[:, :RPC] = xTfull[:, rows]
        ohTc = np.zeros((P, RPAD), np.float32)
        ohTc[:, :RPC] = onehot[:, rows]
        in_maps.append(
            {"xT": _to_mm_np(xTc, mode), "ohT": _to_mm_np(ohTc, mode), **shared}
        )

    global _LAST_IN_MAPS
    _LAST_IN_MAPS = in_maps

    from concourse import bass_utils

    res = bass_utils.run_bass_kernel_spmd(nc, in_maps, core_ids=list(range(NCORES)))
    out = np.empty((N, OUT), np.float32)
    for c in range(NCORES):
        out[c * RPC : (c + 1) * RPC] = res.results[c]["yT"][:, :RPC].T
    return out


# revision 16
# speedup vs baseline: 1.8351x; 1.8351x over previous
"""Trainium2 Bass kernel for the Clusteror GNN message-passing block.

Full (unsharded) inputs in, full output out. Internally: data-parallel over
node rows across 8 NeuronCores; the P=10 virtual-node path is precomputed on
host (it is tiny and feeds every row via a gather, which we lower to a
one-hot matmul).

Math notes (host-side rewrites):
 - LayerNorm mean removal is folded into the weights: column-centered W makes
   every row of z = x@Wc have exactly zero channel-mean, so the device only
   needs the second moment. rstd = exp(-0.5*ln(ssq/C + eps)) keeps all ACT
   work inside one activation-table set (ln+exp+square+identity).
 - The per-row sum of squares is computed with a ones-MATRIX matmul, which
   lands in PSUM already broadcast across all 128 partitions.
 - ELU is computed in a shifted basis: elu(x) + 1 = max(x,0) + min(exp(x),1).
   The -1 shift is folded into the next layer's bias via column sums of its
   weights.
 - The irregular gather v_x[mapping] becomes onehot(mapping) @ (v_x @ Wa2c),
   a K=10 matmul against a host-built one-hot operand.
"""

import json
import os
import tempfile
from pathlib import Path

import numpy as np

N = 200000
P = 10
C = 256
OUT = 64
NCORES = 8
RPC = N // NCORES          # 25000 real rows per core
TILE = int(os.environ.get("KERNEL_TILE", "512"))
NT = (RPC + TILE - 1) // TILE
RPAD = NT * TILE           # 25088 padded rows per core (for TILE in {256,512})
KC = 2                     # 256 channels = 2 partition chunks of 128
EPS = 1e-5

# dtype config: "bf16" (fast) or "f32r" (precise).
MM_MODE = os.environ.get("KERNEL_MM_MODE", "bf16")
PSZ = int(os.environ.get("KERNEL_PSZ", "2"))
PSS = int(os.environ.get("KERNEL_PSS", "2"))
PSY = int(os.environ.get("KERNEL_PSY", "2"))
ABUFS = int(os.environ.get("KERNEL_ABUFS", "3"))

_CACHE = {}
_LAST_IN_MAPS = None


def _np_layer_norm(x, g, b, eps=EPS):
    mu = x.mean(-1, keepdims=True)
    var = x.var(-1, keepdims=True)
    return (x - mu) / np.sqrt(var + eps) * g + b


def _np_elu(x):
    return np.where(x > 0, x, np.expm1(np.minimum(x, 0.0)))


def _ensure_act_root():
    """Reorder act_info.json so natural_log_exp_and_others (which covers
    square+ln+exp+identity+copy) is set 0, and make BOTH bass (which
    pre-places InstLoadActFuncSet with a set id) and walrus (which adopts
    those ids against --act-root-json) see the same reordered file. With a
    covering set first, every ACTIVATE resolves to one table."""
    cur = os.environ.get("BASS_ACT_ROOT_JSON_PATH", "")
    if cur and "_ntbl" in cur:
        return
    import neuronxcc

    src = Path(neuronxcc.__path__[0]) / "pwp" / "pwp_bin_trainium"
    info = json.load(open(src / "act_info.json"))
    sets = info["act_func_sets"]
    tgt = [s for s in sets if s["name"] == "natural_log_exp_and_others"]
    if not tgt:
        return
    info["act_func_sets"] = tgt + [s for s in sets if s is not tgt[0]]
    d = Path(tempfile.mkdtemp(prefix="actroot_ntbl"))
    for f in src.iterdir():
        if f.name != "act_info.json":
            (d / f.name).symlink_to(f)
    act_path = d / "act_info.json"
    with open(act_path, "w") as fh:
        json.dump(info, fh)
    os.environ["BASS_ACT_ROOT_JSON_PATH"] = str(act_path)

    # bass side: get_activation_tables reads the stock act_info via
    # findActInfoFile (cached); repoint it at the reordered file so the
    # pre-placed set ids match what walrus will read.
    import functools

    import concourse.bacc as bacc_mod
    import concourse.hw_specs as hw_specs
    from concourse import mybir

    @functools.cache
    def _tables(module_arch):
        with open(act_path) as af:
            ai = json.load(af)
        return {
            ent["name"]: {
                mybir.ActivationFunctionType.from_pwp(v) for v in ent["act"].keys()
            }
            for ent in ai["act_func_sets"]
        }

    hw_specs.get_activation_tables = _tables
    bacc_mod.get_activation_tables = _tables


def _build(affine1, affine3, b1_zero, mode):
    import concourse.bacc as bacc
    import concourse.tile as tile
    from concourse import mybir

    f32 = mybir.dt.float32
    mdt = mybir.dt.bfloat16 if mode == "bf16" else mybir.dt.float32r
    adt = f32  # bf16 elementwise falls off the DVE fast paths in this stack
    Alu = mybir.AluOpType
    Act = mybir.ActivationFunctionType

    nc = bacc.Bacc("TRN2", target_bir_lowering=False, debug=False)

    # ---- DRAM I/O ----
    xT = nc.dram_tensor("xT", [C, RPAD], mdt, kind="ExternalInput").ap()
    ohT = nc.dram_tensor("ohT", [P, RPAD], mdt, kind="ExternalInput").ap()
    w1 = nc.dram_tensor("w1", [C, C], mdt, kind="ExternalInput").ap()
    w2 = nc.dram_tensor("w2", [C, C], mdt, kind="ExternalInput").ap()
    w3 = nc.dram_tensor("w3", [C, C], mdt, kind="ExternalInput").ap()
    w4 = nc.dram_tensor("w4", [C, OUT], mdt, kind="ExternalInput").ap()
    vaT = nc.dram_tensor("vaT", [P, C], mdt, kind="ExternalInput").ap()
    b1 = nc.dram_tensor("b1", [C, 1], f32, kind="ExternalInput").ap()
    b2 = nc.dram_tensor("b2", [C, 1], f32, kind="ExternalInput").ap()
    b3 = nc.dram_tensor("b3", [C, 1], f32, kind="ExternalInput").ap()
    b4 = nc.dram_tensor("b4", [OUT, 1], f32, kind="ExternalInput").ap()
    gb1 = nc.dram_tensor("gb1", [C, 2], f32, kind="ExternalInput").ap()
    gb3 = nc.dram_tensor("gb3", [C, 2], f32, kind="ExternalInput").ap()
    onesd = nc.dram_tensor("onesd", [128, 128], mdt, kind="ExternalInput").ap()
    yT = nc.dram_tensor("yT", [OUT, RPAD], f32, kind="ExternalOutput").ap()

    with tile.TileContext(nc) as tc:
        with (
            tc.tile_pool(name="wpool", bufs=1) as wp,
            tc.tile_pool(name="xin", bufs=3) as xin,
            tc.tile_pool(name="act", bufs=ABUFS) as act,
            tc.tile_pool(name="small", bufs=2) as sm,
            tc.tile_pool(name="psz", bufs=PSZ, space="PSUM") as psz,
            tc.tile_pool(name="pss", bufs=PSS, space="PSUM") as pss,
            tc.tile_pool(name="psy", bufs=PSY, space="PSUM") as psy,
        ):
            # ---- resident weights/constants ----
            w1t = wp.tile([128, KC, C], mdt)
            nc.sync.dma_start(w1t[:], w1.rearrange("(k p) m -> p k m", p=128))
            w2t = wp.tile([128, KC, C], mdt)
            nc.sync.dma_start(w2t[:], w2.rearrange("(k p) m -> p k m", p=128))
            w3t = wp.tile([128, KC, C], mdt)
            nc.sync.dma_start(w3t[:], w3.rearrange("(k p) m -> p k m", p=128))
            w4t = wp.tile([128, KC, OUT], mdt)
            nc.sync.dma_start(w4t[:], w4.rearrange("(k p) m -> p k m", p=128))
            vat = wp.tile([P, C], mdt)
            nc.sync.dma_start(vat[:], vaT[:])
            b1t = wp.tile([128, KC], f32)
            nc.sync.dma_start(b1t[:], b1.rearrange("(k p) 1 -> p k", p=128))
            b2t = wp.tile([128, KC], f32)
            nc.sync.dma_start(b2t[:], b2.rearrange("(k p) 1 -> p k", p=128))
            b3t = wp.tile([128, KC], f32)
            nc.sync.dma_start(b3t[:], b3.rearrange("(k p) 1 -> p k", p=128))
            b4t = wp.tile([OUT, 1], f32)
            nc.sync.dma_start(b4t[:], b4[:])
            gb1t = wp.tile([128, KC, 2], f32)
            nc.sync.dma_start(gb1t[:], gb1.rearrange("(k p) a -> p k a", p=128))
            gb3t = wp.tile([128, KC, 2], f32)
            nc.sync.dma_start(gb3t[:], gb3.rearrange("(k p) a -> p k a", p=128))
            ones = wp.tile([128, 128], mdt)
            nc.sync.dma_start(ones[:], onesd[:])
            epst = wp.tile([128, 1], f32)
            nc.vector.memset(epst[:], EPS)

            def tile_stages(t, par):
                sfx = str(par)
                cols = slice(t * TILE, (t + 1) * TILE)
                xt = xin.tile([128, KC, TILE], mdt, tag="xt" + sfx)
                nc.sync.dma_start(
                    xt[:], xT[:, cols].rearrange("(k p) n -> p k n", p=128)
                )
                oht = xin.tile([P, TILE], mdt, tag="oht" + sfx)
                nc.sync.dma_start(oht[:], ohT[:, cols])
                yield
                # L1
                z1 = psz.tile([128, KC, TILE], f32, tag="z" + sfx)
                for m in range(KC):
                    for k in range(KC):
                        nc.tensor.matmul(
                            z1[:, m, :],
                            w1t[:, k, m * 128 : (m + 1) * 128],
                            xt[:, k, :],
                            start=(k == 0), stop=(k == KC - 1),
                        )
                yield
                result = []
                yield from ln_elu_stages_wrap(
                    z1, b1t, b1_zero, gb1t, affine1, sfx, result
                )
                h1 = result[0]
                # L2
                z2 = psz.tile([128, KC, TILE], f32, tag="z" + sfx)
                for m in range(KC):
                    for k in range(KC):
                        nc.tensor.matmul(
                            z2[:, m, :],
                            w2t[:, k, m * 128 : (m + 1) * 128],
                            h1[:, k, :],
                            start=(k == 0), stop=(k == KC - 1),
                        )
                yield
                ex = act.tile([128, KC, TILE], adt, tag="ex" + sfx)
                r2 = act.tile([128, KC, TILE], adt, tag="r2" + sfx)
                for c in range(KC):
                    nc.scalar.activation(
                        ex[:, c, :], z2[:, c, :], Act.Exp, bias=b2t[:, c : c + 1]
                    )
                    nc.vector.tensor_scalar(
                        r2[:, c, :], z2[:, c, :], b2t[:, c : c + 1], 0.0,
                        op0=Alu.add, op1=Alu.max,
                    )
                yield
                h2 = act.tile([128, KC, TILE], mdt, tag="h2" + sfx)
                nc.vector.scalar_tensor_tensor(
                    h2[:, :, :], ex[:, :, :], 1.0, r2[:, :, :],
                    op0=Alu.min, op1=Alu.add,
                )
                yield
                # L3
                z3 = psz.tile([128, KC, TILE], f32, tag="z" + sfx)
                for m in range(KC):
                    for k in range(KC):
                        nc.tensor.matmul(
                            z3[:, m, :],
                            w3t[:, k, m * 128 : (m + 1) * 128],
                            h2[:, k, :],
                            start=(k == 0), stop=False,
                        )
                    nc.tensor.matmul(
                        z3[:, m, :],
                        vat[:, m * 128 : (m + 1) * 128],
                        oht[:],
                        start=False, stop=True,
                    )
                yield
                result = []
                yield from ln_elu_stages_wrap(
                    z3, b3t, False, gb3t, affine3, sfx, result
                )
                h3 = result[0]
                # L4
                y = psy.tile([OUT, TILE], f32, tag="y" + sfx)
                for k in range(KC):
                    nc.tensor.matmul(
                        y[:], w4t[:, k, :], h3[:, k, :],
                        start=(k == 0), stop=(k == KC - 1),
                    )
                yield
                yb = sm.tile([OUT, TILE], f32, tag="yb" + sfx)
                nc.scalar.activation(yb[:], y[:], Act.Identity, bias=b4t[:])
                nc.sync.dma_start(yT[:, cols], yb[:])

            def ln_elu_stages_wrap(z, bt, b_zero, gbt, affine, sfx, result):
                zsq = act.tile([128, KC, TILE], mdt, tag="zsq" + sfx)
                if b_zero:
                    nc.scalar.activation(zsq[:, :, :], z[:, :, :], Act.Square)
                else:
                    for c in range(KC):
                        nc.scalar.activation(
                            zsq[:, c, :], z[:, c, :], Act.Square,
                            bias=bt[:, c : c + 1],
                        )
                yield
                ssb = pss.tile([128, TILE], f32, tag="ssb" + sfx)
                for c in range(KC):
                    nc.tensor.matmul(
                        ssb[:], ones[:], zsq[:, c, :],
                        start=(c == 0), stop=(c == KC - 1),
                    )
                yield
                lg = sm.tile([128, TILE], f32, tag="lg" + sfx)
                nc.scalar.activation(lg[:], ssb[:], Act.Ln, scale=1.0 / C, bias=epst[:])
                rstd = sm.tile([128, TILE], f32, tag="rstd" + sfx)
                nc.scalar.activation(rstd[:], lg[:], Act.Exp, scale=-0.5)
                yield
                u = act.tile([128, KC, TILE], adt, tag="u" + sfx)
                for c in range(KC):
                    nc.vector.scalar_tensor_tensor(
                        u[:, c, :], z[:, c, :], bt[:, c : c + 1], rstd[:],
                        op0=Alu.add, op1=Alu.mult,
                    )
                    if affine:
                        nc.vector.tensor_scalar(
                            u[:, c, :], u[:, c, :], gbt[:, c, 0:1], gbt[:, c, 1:2],
                            op0=Alu.mult, op1=Alu.add,
                        )
                yield
                e = act.tile([128, KC, TILE], adt, tag="e" + sfx)
                nc.scalar.activation(e[:, :, :], u[:, :, :], Act.Exp)
                r = act.tile([128, KC, TILE], adt, tag="r" + sfx)
                nc.vector.tensor_scalar(r[:, :, :], u[:, :, :], 0.0, None, op0=Alu.max)
                h = act.tile([128, KC, TILE], mdt, tag="h" + sfx)
                nc.vector.scalar_tensor_tensor(
                    h[:, :, :], e[:, :, :], 1.0, r[:, :, :], op0=Alu.min, op1=Alu.add
                )
                result.append(h)

            # drive pairs of tiles through the stages in lockstep so each
            # engine's queue alternates between two independent chains
            t = 0
            while t < NT:
                if t + 1 < NT:
                    g0 = tile_stages(t, 0)
                    g1 = tile_stages(t + 1, 1)
                    done0 = done1 = False
                    while not (done0 and done1):
                        if not done0:
                            try:
                                next(g0)
                            except StopIteration:
                                done0 = True
                        if not done1:
                            try:
                                next(g1)
                            except StopIteration:
                                done1 = True
                    t += 2
                else:
                    for _ in tile_stages(t, 0):
                        pass
                    t += 1

    nc.compile()
    return nc


def _to_mm_np(a, mode):
    if mode == "bf16":
        import ml_dtypes

        return np.ascontiguousarray(a, dtype=ml_dtypes.bfloat16)
    return np.ascontiguousarray(a, np.float32)


def kernel(**inputs):
    _ensure_act_root()
    x = np.asarray(inputs["x"], np.float32)
    mapping = np.asarray(inputs["mapping"])
    vnode_embed = np.asarray(inputs["vnode_embed"], np.float32)
    vnode_bias_hid = np.asarray(inputs["vnode_bias_hid"], np.float32)
    vnode_bias_dcd = np.asarray(inputs["vnode_bias_dcd"], np.float32)
    W1 = np.asarray(inputs["W_in2hid"], np.float32)
    b1_in = np.asarray(inputs["b_in2hid"], np.float32)
    g_hid = np.asarray(inputs["g_ln_hid"], np.float32)
    b_hid = np.asarray(inputs["b_ln_hid"], np.float32)
    W2 = np.asarray(inputs["W_enc"], np.float32)
    b2_in = np.asarray(inputs["b_enc"], np.float32)
    g_enc = np.asarray(inputs["g_ln_enc"], np.float32)
    b_enc_ln = np.asarray(inputs["b_ln_enc"], np.float32)
    Wa = np.asarray(inputs["W_aggr"], np.float32)
    b_aggr = np.asarray(inputs["b_aggr"], np.float32)
    W4 = np.asarray(inputs["W_out"], np.float32)
    b_out = np.asarray(inputs["b_out"], np.float32)
    mode = MM_MODE

    # ---- host-side weight prep (float64 for the centering arithmetic) ----
    W1d = W1.astype(np.float64)
    W1c = W1d - W1d.mean(1, keepdims=True)
    b1c = b1_in.astype(np.float64) - b1_in.astype(np.float64).mean()

    W2d = W2.astype(np.float64)
    bias2 = b2_in.astype(np.float64) - W2d.sum(0)

    Wa1 = Wa[:C].astype(np.float64)
    Wa2 = Wa[C:].astype(np.float64)
    Wa1c = Wa1 - Wa1.mean(1, keepdims=True)
    Wa2c = Wa2 - Wa2.mean(1, keepdims=True)
    bad = b_aggr.astype(np.float64)
    bias3 = (bad - bad.mean()) - Wa1c.sum(0)

    W4d = W4.astype(np.float64)
    bias4 = b_out.astype(np.float64) - W4d.sum(0)

    affine1 = not (np.all(g_hid == 1.0) and np.all(b_hid == 0.0))
    affine3 = not (np.all(g_enc == 1.0) and np.all(b_enc_ln == 0.0))
    b1_zero = bool(np.all(np.asarray(b1c) == 0.0))

    # ---- virtual-node path entirely on host (10 rows) ----
    v0 = vnode_embed.astype(np.float64)
    hv = _np_elu(_np_layer_norm(v0 @ W1d + b1_in, g_hid, b_hid)) + vnode_bias_hid
    ev = _np_elu(hv @ W2d + b2_in) + vnode_bias_dcd   # true v_x [P, C]
    va_c = ev @ Wa2c                                   # centered gather payload

    # ---- shard + transpose real-node rows ----
    onehot = (mapping[None, :] == np.arange(P, dtype=mapping.dtype)[:, None])
    onehot = onehot.astype(np.float32)

    key = (affine1, affine3, b1_zero, mode, TILE, PSZ, PSS, PSY, ABUFS)
    if key not in _CACHE:
        _CACHE[key] = _build(affine1, affine3, b1_zero, mode)
    nc = _CACHE[key]

    shared = {
        "w1": _to_mm_np(W1c, mode),
        "w2": _to_mm_np(W2d, mode),
        "w3": _to_mm_np(Wa1c, mode),
        "w4": _to_mm_np(W4d, mode),
        "vaT": _to_mm_np(va_c, mode),
        "b1": np.ascontiguousarray(b1c, np.float32).reshape(C, 1),
        "b2": np.ascontiguousarray(bias2, np.float32).reshape(C, 1),
        "b3": np.ascontiguousarray(bias3, np.float32).reshape(C, 1),
        "b4": np.ascontiguousarray(bias4, np.float32).reshape(OUT, 1),
        "gb1": np.stack([g_hid, b_hid], 1).astype(np.float32),
        "gb3": np.stack([g_enc, b_enc_ln], 1).astype(np.float32),
        "onesd": _to_mm_np(np.ones((128, 128), np.float32), mode),
    }
    in_maps = []
    xTfull = x[:N].T  # [C, N]
    for c in range(NCORES):
        rows = slice(c * RPC, (c + 1) * RPC)
        xTc = np.zeros((C, RPAD), np.float32)
        xTc[:, :RPC] = xTfull[:, rows]
        ohTc = np.zeros((P, RPAD), np.float32)
        ohTc[:, :RPC] = onehot[:, rows]
        in_maps.append(
            {"xT": _to_mm_np(xTc, mode), "ohT": _to_mm_np(ohTc, mode), **shared}
        )

    global _LAST_IN_MAPS
    _LAST_IN_MAPS = in_maps

    from concourse import bass_utils

    res = bass_utils.run_bass_kernel_spmd(nc, in_maps, core_ids=list(range(NCORES)))
    out = np.empty((N, OUT), np.float32)
    for c in range(NCORES):
        out[c * RPC : (c + 1) * RPC] = res.results[c]["yT"][:, :RPC].T
    return out


# revision 17
# speedup vs baseline: 2.0396x; 1.1115x over previous
"""Trainium2 Bass kernel for the Clusteror GNN message-passing block.

Full (unsharded) inputs in, full output out. Internally: data-parallel over
node rows across 8 NeuronCores; the P=10 virtual-node path is precomputed on
host (it is tiny and feeds every row via a gather, which we lower to a
one-hot matmul).

Math notes (host-side rewrites):
 - LayerNorm mean removal is folded into the weights: column-centered W makes
   every row of z = x@Wc have exactly zero channel-mean, so the device only
   needs the second moment. rstd = exp(-0.5*ln(ssq/C + eps)) keeps all ACT
   work inside one activation-table set (ln+exp+square+identity).
 - The per-row sum of squares is computed with a ones-MATRIX matmul, which
   lands in PSUM already broadcast across all 128 partitions.
 - ELU is computed in a shifted basis: elu(x) + 1 = max(x,0) + min(exp(x),1).
   The -1 shift is folded into the next layer's bias via column sums of its
   weights.
 - The irregular gather v_x[mapping] becomes onehot(mapping) @ (v_x @ Wa2c),
   a K=10 matmul against a host-built one-hot operand.
"""

import json
import os
import tempfile
from pathlib import Path

import numpy as np

N = 200000
P = 10
C = 256
OUT = 64
NCORES = 8
RPC = N // NCORES          # 25000 real rows per core
TILE = int(os.environ.get("KERNEL_TILE", "512"))
NT = (RPC + TILE - 1) // TILE
RPAD = NT * TILE           # 25088 padded rows per core (for TILE in {256,512})
KC = 2                     # 256 channels = 2 partition chunks of 128
EPS = 1e-5

# dtype config: "bf16" (fast) or "f32r" (precise).
MM_MODE = os.environ.get("KERNEL_MM_MODE", "bf16")
NSTREAM = int(os.environ.get("KERNEL_NSTREAM", "3"))
PSZ = int(os.environ.get("KERNEL_PSZ", "2"))
PSS = int(os.environ.get("KERNEL_PSS", "2"))
PSY = int(os.environ.get("KERNEL_PSY", "2"))
ABUFS = int(os.environ.get("KERNEL_ABUFS", "3"))

_CACHE = {}
_LAST_IN_MAPS = None


def _np_layer_norm(x, g, b, eps=EPS):
    mu = x.mean(-1, keepdims=True)
    var = x.var(-1, keepdims=True)
    return (x - mu) / np.sqrt(var + eps) * g + b


def _np_elu(x):
    return np.where(x > 0, x, np.expm1(np.minimum(x, 0.0)))


def _ensure_act_root():
    """Reorder act_info.json so natural_log_exp_and_others (which covers
    square+ln+exp+identity+copy) is set 0, and make BOTH bass (which
    pre-places InstLoadActFuncSet with a set id) and walrus (which adopts
    those ids against --act-root-json) see the same reordered file. With a
    covering set first, every ACTIVATE resolves to one table."""
    cur = os.environ.get("BASS_ACT_ROOT_JSON_PATH", "")
    if cur and "_ntbl" in cur:
        return
    import neuronxcc

    src = Path(neuronxcc.__path__[0]) / "pwp" / "pwp_bin_trainium"
    info = json.load(open(src / "act_info.json"))
    sets = info["act_func_sets"]
    tgt = [s for s in sets if s["name"] == "natural_log_exp_and_others"]
    if not tgt:
        return
    info["act_func_sets"] = tgt + [s for s in sets if s is not tgt[0]]
    d = Path(tempfile.mkdtemp(prefix="actroot_ntbl"))
    for f in src.iterdir():
        if f.name != "act_info.json":
            (d / f.name).symlink_to(f)
    act_path = d / "act_info.json"
    with open(act_path, "w") as fh:
        json.dump(info, fh)
    os.environ["BASS_ACT_ROOT_JSON_PATH"] = str(act_path)

    # bass side: get_activation_tables reads the stock act_info via
    # findActInfoFile (cached); repoint it at the reordered file so the
    # pre-placed set ids match what walrus will read.
    import functools

    import concourse.bacc as bacc_mod
    import concourse.hw_specs as hw_specs
    from concourse import mybir

    @functools.cache
    def _tables(module_arch):
        with open(act_path) as af:
            ai = json.load(af)
        return {
            ent["name"]: {
                mybir.ActivationFunctionType.from_pwp(v) for v in ent["act"].keys()
            }
            for ent in ai["act_func_sets"]
        }

    hw_specs.get_activation_tables = _tables
    bacc_mod.get_activation_tables = _tables


def _build(affine1, affine3, b1_zero, mode):
    import concourse.bacc as bacc
    import concourse.tile as tile
    from concourse import mybir

    f32 = mybir.dt.float32
    mdt = mybir.dt.bfloat16 if mode == "bf16" else mybir.dt.float32r
    adt = f32  # bf16 elementwise falls off the DVE fast paths in this stack
    Alu = mybir.AluOpType
    Act = mybir.ActivationFunctionType

    nc = bacc.Bacc("TRN2", target_bir_lowering=False, debug=False)

    # ---- DRAM I/O ----
    xT = nc.dram_tensor("xT", [C, RPAD], mdt, kind="ExternalInput").ap()
    ohT = nc.dram_tensor("ohT", [P, RPAD], mdt, kind="ExternalInput").ap()
    w1 = nc.dram_tensor("w1", [C, C], mdt, kind="ExternalInput").ap()
    w2 = nc.dram_tensor("w2", [C, C], mdt, kind="ExternalInput").ap()
    w3 = nc.dram_tensor("w3", [C, C], mdt, kind="ExternalInput").ap()
    w4 = nc.dram_tensor("w4", [C, OUT], mdt, kind="ExternalInput").ap()
    vaT = nc.dram_tensor("vaT", [P, C], mdt, kind="ExternalInput").ap()
    b1 = nc.dram_tensor("b1", [C, 1], f32, kind="ExternalInput").ap()
    b2 = nc.dram_tensor("b2", [C, 1], f32, kind="ExternalInput").ap()
    b3 = nc.dram_tensor("b3", [C, 1], f32, kind="ExternalInput").ap()
    b4 = nc.dram_tensor("b4", [OUT, 1], f32, kind="ExternalInput").ap()
    gb1 = nc.dram_tensor("gb1", [C, 2], f32, kind="ExternalInput").ap()
    gb3 = nc.dram_tensor("gb3", [C, 2], f32, kind="ExternalInput").ap()
    onesd = nc.dram_tensor("onesd", [128, 128], mdt, kind="ExternalInput").ap()
    yT = nc.dram_tensor("yT", [OUT, RPAD], f32, kind="ExternalOutput").ap()

    with tile.TileContext(nc) as tc:
        with (
            tc.tile_pool(name="wpool", bufs=1) as wp,
            tc.tile_pool(name="xin", bufs=3) as xin,
            tc.tile_pool(name="act", bufs=ABUFS) as act,
            tc.tile_pool(name="small", bufs=2) as sm,
            tc.tile_pool(name="psz", bufs=PSZ, space="PSUM") as psz,
            tc.tile_pool(name="pss", bufs=PSS, space="PSUM") as pss,
            tc.tile_pool(name="psy", bufs=PSY, space="PSUM") as psy,
        ):
            # ---- resident weights/constants ----
            w1t = wp.tile([128, KC, C], mdt)
            nc.sync.dma_start(w1t[:], w1.rearrange("(k p) m -> p k m", p=128))
            w2t = wp.tile([128, KC, C], mdt)
            nc.sync.dma_start(w2t[:], w2.rearrange("(k p) m -> p k m", p=128))
            w3t = wp.tile([128, KC, C], mdt)
            nc.sync.dma_start(w3t[:], w3.rearrange("(k p) m -> p k m", p=128))
            w4t = wp.tile([128, KC, OUT], mdt)
            nc.sync.dma_start(w4t[:], w4.rearrange("(k p) m -> p k m", p=128))
            vat = wp.tile([P, C], mdt)
            nc.sync.dma_start(vat[:], vaT[:])
            b1t = wp.tile([128, KC], f32)
            nc.sync.dma_start(b1t[:], b1.rearrange("(k p) 1 -> p k", p=128))
            b2t = wp.tile([128, KC], f32)
            nc.sync.dma_start(b2t[:], b2.rearrange("(k p) 1 -> p k", p=128))
            b3t = wp.tile([128, KC], f32)
            nc.sync.dma_start(b3t[:], b3.rearrange("(k p) 1 -> p k", p=128))
            b4t = wp.tile([OUT, 1], f32)
            nc.sync.dma_start(b4t[:], b4[:])
            gb1t = wp.tile([128, KC, 2], f32)
            nc.sync.dma_start(gb1t[:], gb1.rearrange("(k p) a -> p k a", p=128))
            gb3t = wp.tile([128, KC, 2], f32)
            nc.sync.dma_start(gb3t[:], gb3.rearrange("(k p) a -> p k a", p=128))
            ones = wp.tile([128, 128], mdt)
            nc.sync.dma_start(ones[:], onesd[:])
            epst = wp.tile([128, 1], f32)
            nc.vector.memset(epst[:], EPS)

            def tile_stages(t, par):
                sfx = str(par)
                cols = slice(t * TILE, (t + 1) * TILE)
                xt = xin.tile([128, KC, TILE], mdt, tag="xt" + sfx)
                nc.sync.dma_start(
                    xt[:], xT[:, cols].rearrange("(k p) n -> p k n", p=128)
                )
                oht = xin.tile([P, TILE], mdt, tag="oht" + sfx)
                nc.sync.dma_start(oht[:], ohT[:, cols])
                yield
                # L1
                z1 = psz.tile([128, KC, TILE], f32, tag="z" + sfx)
                for m in range(KC):
                    for k in range(KC):
                        nc.tensor.matmul(
                            z1[:, m, :],
                            w1t[:, k, m * 128 : (m + 1) * 128],
                            xt[:, k, :],
                            start=(k == 0), stop=(k == KC - 1),
                        )
                yield
                result = []
                yield from ln_elu_stages_wrap(
                    z1, b1t, b1_zero, gb1t, affine1, sfx, result
                )
                h1 = result[0]
                # L2
                z2 = psz.tile([128, KC, TILE], f32, tag="z" + sfx)
                for m in range(KC):
                    for k in range(KC):
                        nc.tensor.matmul(
                            z2[:, m, :],
                            w2t[:, k, m * 128 : (m + 1) * 128],
                            h1[:, k, :],
                            start=(k == 0), stop=(k == KC - 1),
                        )
                yield
                ex = act.tile([128, KC, TILE], adt, tag="ex" + sfx)
                r2 = act.tile([128, KC, TILE], adt, tag="r2" + sfx)
                for c in range(KC):
                    nc.scalar.activation(
                        ex[:, c, :], z2[:, c, :], Act.Exp, bias=b2t[:, c : c + 1]
                    )
                    nc.vector.tensor_scalar(
                        r2[:, c, :], z2[:, c, :], b2t[:, c : c + 1], 0.0,
                        op0=Alu.add, op1=Alu.max,
                    )
                yield
                h2 = act.tile([128, KC, TILE], mdt, tag="h2" + sfx)
                nc.vector.scalar_tensor_tensor(
                    h2[:, :, :], ex[:, :, :], 1.0, r2[:, :, :],
                    op0=Alu.min, op1=Alu.add,
                )
                yield
                # L3
                z3 = psz.tile([128, KC, TILE], f32, tag="z" + sfx)
                for m in range(KC):
                    for k in range(KC):
                        nc.tensor.matmul(
                            z3[:, m, :],
                            w3t[:, k, m * 128 : (m + 1) * 128],
                            h2[:, k, :],
                            start=(k == 0), stop=False,
                        )
                    nc.tensor.matmul(
                        z3[:, m, :],
                        vat[:, m * 128 : (m + 1) * 128],
                        oht[:],
                        start=False, stop=True,
                    )
                yield
                result = []
                yield from ln_elu_stages_wrap(
                    z3, b3t, False, gb3t, affine3, sfx, result
                )
                h3 = result[0]
                # L4
                y = pss.tile([OUT, TILE], f32, tag="sy")
                for k in range(KC):
                    nc.tensor.matmul(
                        y[:], w4t[:, k, :], h3[:, k, :],
                        start=(k == 0), stop=(k == KC - 1),
                    )
                yield
                yb = sm.tile([OUT, TILE], f32, tag="yb" + sfx)
                nc.scalar.activation(yb[:], y[:], Act.Identity, bias=b4t[:])
                nc.sync.dma_start(yT[:, cols], yb[:])

            def ln_elu_stages_wrap(z, bt, b_zero, gbt, affine, sfx, result):
                zsq = act.tile([128, KC, TILE], mdt, tag="zsq" + sfx)
                if b_zero:
                    nc.scalar.activation(zsq[:, :, :], z[:, :, :], Act.Square)
                else:
                    for c in range(KC):
                        nc.scalar.activation(
                            zsq[:, c, :], z[:, c, :], Act.Square,
                            bias=bt[:, c : c + 1],
                        )
                yield
                ssb = pss.tile([128, TILE], f32, tag="sy")
                for c in range(KC):
                    nc.tensor.matmul(
                        ssb[:], ones[:], zsq[:, c, :],
                        start=(c == 0), stop=(c == KC - 1),
                    )
                yield
                lg = sm.tile([128, TILE], f32, tag="lg" + sfx)
                nc.scalar.activation(lg[:], ssb[:], Act.Ln, scale=1.0 / C, bias=epst[:])
                rstd = sm.tile([128, TILE], f32, tag="rstd" + sfx)
                nc.scalar.activation(rstd[:], lg[:], Act.Exp, scale=-0.5)
                yield
                u = act.tile([128, KC, TILE], adt, tag="u" + sfx)
                for c in range(KC):
                    nc.vector.scalar_tensor_tensor(
                        u[:, c, :], z[:, c, :], bt[:, c : c + 1], rstd[:],
                        op0=Alu.add, op1=Alu.mult,
                    )
                    if affine:
                        nc.vector.tensor_scalar(
                            u[:, c, :], u[:, c, :], gbt[:, c, 0:1], gbt[:, c, 1:2],
                            op0=Alu.mult, op1=Alu.add,
                        )
                yield
                e = act.tile([128, KC, TILE], adt, tag="e" + sfx)
                nc.scalar.activation(e[:, :, :], u[:, :, :], Act.Exp)
                r = act.tile([128, KC, TILE], adt, tag="r" + sfx)
                nc.vector.tensor_scalar(r[:, :, :], u[:, :, :], 0.0, None, op0=Alu.max)
                h = act.tile([128, KC, TILE], mdt, tag="h" + sfx)
                nc.vector.scalar_tensor_tensor(
                    h[:, :, :], e[:, :, :], 1.0, r[:, :, :], op0=Alu.min, op1=Alu.add
                )
                result.append(h)

            # drive NSTREAM tiles through the stages in lockstep so each
            # engine's queue interleaves independent chains (fills
            # cross-engine stalls, keeps PE bursts dense for HAM warmth)
            t = 0
            while t < NT:
                n = min(NSTREAM, NT - t)
                gens = [tile_stages(t + i, i) for i in range(n)]
                done = [False] * n
                while not all(done):
                    for i, g in enumerate(gens):
                        if not done[i]:
                            try:
                                next(g)
                            except StopIteration:
                                done[i] = True
                t += n

    nc.compile()
    return nc


def _to_mm_np(a, mode):
    if mode == "bf16":
        import ml_dtypes

        return np.ascontiguousarray(a, dtype=ml_dtypes.bfloat16)
    return np.ascontiguousarray(a, np.float32)


def kernel(**inputs):
    _ensure_act_root()
    x = np.asarray(inputs["x"], np.float32)
    mapping = np.asarray(inputs["mapping"])
    vnode_embed = np.asarray(inputs["vnode_embed"], np.float32)
    vnode_bias_hid = np.asarray(inputs["vnode_bias_hid"], np.float32)
    vnode_bias_dcd = np.asarray(inputs["vnode_bias_dcd"], np.float32)
    W1 = np.asarray(inputs["W_in2hid"], np.float32)
    b1_in = np.asarray(inputs["b_in2hid"], np.float32)
    g_hid = np.asarray(inputs["g_ln_hid"], np.float32)
    b_hid = np.asarray(inputs["b_ln_hid"], np.float32)
    W2 = np.asarray(inputs["W_enc"], np.float32)
    b2_in = np.asarray(inputs["b_enc"], np.float32)
    g_enc = np.asarray(inputs["g_ln_enc"], np.float32)
    b_enc_ln = np.asarray(inputs["b_ln_enc"], np.float32)
    Wa = np.asarray(inputs["W_aggr"], np.float32)
    b_aggr = np.asarray(inputs["b_aggr"], np.float32)
    W4 = np.asarray(inputs["W_out"], np.float32)
    b_out = np.asarray(inputs["b_out"], np.float32)
    mode = MM_MODE

    # ---- host-side weight prep (float64 for the centering arithmetic) ----
    W1d = W1.astype(np.float64)
    W1c = W1d - W1d.mean(1, keepdims=True)
    b1c = b1_in.astype(np.float64) - b1_in.astype(np.float64).mean()

    W2d = W2.astype(np.float64)
    bias2 = b2_in.astype(np.float64) - W2d.sum(0)

    Wa1 = Wa[:C].astype(np.float64)
    Wa2 = Wa[C:].astype(np.float64)
    Wa1c = Wa1 - Wa1.mean(1, keepdims=True)
    Wa2c = Wa2 - Wa2.mean(1, keepdims=True)
    bad = b_aggr.astype(np.float64)
    bias3 = (bad - bad.mean()) - Wa1c.sum(0)

    W4d = W4.astype(np.float64)
    bias4 = b_out.astype(np.float64) - W4d.sum(0)

    affine1 = not (np.all(g_hid == 1.0) and np.all(b_hid == 0.0))
    affine3 = not (np.all(g_enc == 1.0) and np.all(b_enc_ln == 0.0))
    b1_zero = bool(np.all(np.asarray(b1c) == 0.0))

    # ---- virtual-node path entirely on host (10 rows) ----
    v0 = vnode_embed.astype(np.float64)
    hv = _np_elu(_np_layer_norm(v0 @ W1d + b1_in, g_hid, b_hid)) + vnode_bias_hid
    ev = _np_elu(hv @ W2d + b2_in) + vnode_bias_dcd   # true v_x [P, C]
    va_c = ev @ Wa2c                                   # centered gather payload

    # ---- shard + transpose real-node rows ----
    onehot = (mapping[None, :] == np.arange(P, dtype=mapping.dtype)[:, None])
    onehot = onehot.astype(np.float32)

    key = (affine1, affine3, b1_zero, mode, TILE, PSZ, PSS, PSY, ABUFS, NSTREAM)
    if key not in _CACHE:
        _CACHE[key] = _build(affine1, affine3, b1_zero, mode)
    nc = _CACHE[key]

    shared = {
        "w1": _to_mm_np(W1c, mode),
        "w2": _to_mm_np(W2d, mode),
        "w3": _to_mm_np(Wa1c, mode),
        "w4": _to_mm_np(W4d, mode),
        "vaT": _to_mm_np(va_c, mode),
        "b1": np.ascontiguousarray(b1c, np.float32).reshape(C, 1),
        "b2": np.ascontiguousarray(bias2, np.float32).reshape(C, 1),
        "b3": np.ascontiguousarray(bias3, np.float32).reshape(C, 1),
        "b4": np.ascontiguousarray(bias4, np.float32).reshape(OUT, 1),
        "gb1": np.stack([g_hid, b_hid], 1).astype(np.float32),
        "gb3": np.stack([g_enc, b_enc_ln], 1).astype(np.float32),
        "onesd": _to_mm_np(np.ones((128, 128), np.float32), mode),
    }
    in_maps = []
    xTfull = x[:N].T  # [C, N]
    for c in range(NCORES):
        rows = slice(c * RPC, (c + 1) * RPC)
        xTc = np.zeros((C, RPAD), np.float32)
        xTc[:, :RPC] = xTfull[:, rows]
        ohTc = np.zeros((P, RPAD), np.float32)
        ohTc[:, :RPC] = onehot[:, rows]
        in_maps.append(
            {"xT": _to_mm_np(xTc, mode), "ohT": _to_mm_np(ohTc, mode), **shared}
        )

    global _LAST_IN_MAPS
    _LAST_IN_MAPS = in_maps

    from concourse import bass_utils

    res = bass_utils.run_bass_kernel_spmd(nc, in_maps, core_ids=list(range(NCORES)))
    out = np.empty((N, OUT), np.float32)
    for c in range(NCORES):
        out[c * RPC : (c + 1) * RPC] = res.results[c]["yT"][:, :RPC].T
    return out
